# revision 7
# baseline (speedup 1.0000x reference)
"""Trainium2 Bass kernel for nn_EncoderLayer_42399917146737.

The reference "SSM scan" is degenerate: at every step i the recurrence
overwrites h at exactly the positions p with pc[p,i]==1 with the scalar
b_i, and the step output reads only those positions.  Hence

    y_i[b] = C[b,i] * Bcoef[b,i] * n_i,      n_i = sum_p pc[p,i]

with no sequential dependence, and the reverse scan equals the forward
one.  The broadcast over p then reduces the Wr projection to a scalar
sum, so the whole module collapses to

    logits[b,l] = 2*sum(Wr) * has_err[b] * n_l * C[b,l] * (Bbias[b,l]/M + tanh(|X[b,l]|*wb_l))
    out         = softmax_l(logits)

where  Bbias = h0 @ pc,  h0 = 1-2*parity(hard @ pc^T),  hard = (X<0),
M = max|Bbias| (GLOBAL over the full batch),  wb = Wb @ pc,  wc = Wc @ pc,
C = 0.5 + tanh(|X|*wc_l).  (br shifts all logits equally -> drops out of
softmax.)

Sharding: batch B=128 over 8 cores (16 rows each).  Because M is a
global max over the whole batch, every core recomputes the (cheap)
full-batch parity/Bbias matmuls; the per-batch elementwise work + softmax
run only on the core's own 16 rows.  Per-core batch selection is done
with a per-core one-hot selection matrix (E_c) fed through the tensor
engine, so a single NEFF serves all 8 cores.

Layouts: the heavy math runs transposed, (L on partitions, batch on the
free axis), so the per-l quantities wb/wc/n are per-partition scalars.
Weight precision: pc is {0,1} so bf16 matmuls with f32 accumulate are
exact; Wb/Wc ride along as bf16 hi+lo split columns (~2^-16 rel err).
"""

import numpy as np
import ml_dtypes

B, L, P = 128, 1024, 512
NCORES = 8
BS = B // NCORES  # 16
LT = L // 128     # 8 L-tiles
PT = P // 128     # 4 P-tiles

_cache = {}


def _build_nc():
    import concourse.bass as bass
    import concourse.bacc as bacc
    import concourse.tile as tile
    from concourse import mybir

    f32 = mybir.dt.float32
    bf16 = mybir.dt.bfloat16
    u32 = mybir.dt.uint32
    Alu = mybir.AluOpType
    Act = mybir.ActivationFunctionType
    Ax = mybir.AxisListType

    nc = bacc.Bacc("TRN2", target_bir_lowering=False, debug=False)

    # ---- DRAM I/O (per-core shapes; host pre-swizzles for contiguous DMA) ----
    xt_d = nc.dram_tensor("xt", (128, LT * 128), f32, kind="ExternalInput")
    xl_d = nc.dram_tensor("xl", (128, LT * BS), f32, kind="ExternalInput")
    pct_d = nc.dram_tensor("pct", (128, LT * P), bf16, kind="ExternalInput")
    pcl_d = nc.dram_tensor("pcl", (128, PT * L), bf16, kind="ExternalInput")
    wt_d = nc.dram_tensor("wt", (128, PT * 2), f32, kind="ExternalInput")
    wr_d = nc.dram_tensor("wr", (1, P), f32, kind="ExternalInput")
    ec_d = nc.dram_tensor("ec", (128, BS), f32, kind="ExternalInput")
    idn_d = nc.dram_tensor("idn", (128, 128), f32, kind="ExternalInput")
    y_d = nc.dram_tensor("y", (BS, L), f32, kind="ExternalOutput")

    NW = 5                    # wb_hi, wc_hi, ones, wb_lo, wc_lo
    NB = 128 + BS             # m^T | m^T_loc
    NR = NB + NW              # combined-matmul rhs width

    def bcast(col_ap, n):
        """Free-dim step-0 broadcast of a (...,1) AP to (...,n)."""
        return bass.AP(tensor=col_ap.tensor, offset=col_ap.offset,
                       ap=[*col_ap.ap[:-1], [0, n]])

    with tile.TileContext(nc) as tc:
        with (
            tc.tile_pool(name="sb", bufs=1) as sb,
            tc.tile_pool(name="ps", bufs=2, space="PSUM") as ps,
            tc.tile_pool(name="ps2", bufs=2, space="PSUM") as ps2,
            tc.tile_pool(name="ps3", bufs=1, space="PSUM") as ps3,
        ):
            XT = sb.tile([128, LT, 128], f32)
            XL = sb.tile([128, LT, BS], f32)
            PCT = sb.tile([128, LT, P], bf16)
            PCL = sb.tile([128, PT, L], bf16)
            WT = sb.tile([128, PT, 2], f32)
            WR = sb.tile([1, P], f32)
            EC = sb.tile([128, BS], f32)
            IDN = sb.tile([128, 128], f32)
            # DMAs: two HWDGE rings (sync + scalar), FIFO order = priority.
            # Ring A: xt half A, pct tiles 0-3, then stragglers.
            # Ring B: xt half B, pct tiles 4-7, then pcl (needed later).
            nc.sync.dma_start(
                XT[:, 0:4, :].rearrange("p i b -> p (i b)"), xt_d[:, 0:512])
            nc.scalar.dma_start(
                XT[:, 4:8, :].rearrange("p i b -> p (i b)"), xt_d[:, 512:1024])
            for i in range(LT):
                eng = nc.sync if i < 4 else nc.scalar
                eng.dma_start(PCT[:, i, :], pct_d[:, i * P:(i + 1) * P])
            nc.sync.dma_start(XL[:].rearrange("p i j -> p (i j)"), xl_d[:])
            nc.sync.dma_start(EC[:], ec_d[:])
            nc.sync.dma_start(WT[:].rearrange("p k t -> p (k t)"), wt_d[:])
            nc.sync.dma_start(WR[:], wr_d[:])
            nc.sync.dma_start(IDN[:], idn_d[:])
            for k in range(PT):
                nc.scalar.dma_start(PCL[:, k, :], pcl_d[:, k * L:(k + 1) * L])

            # ---- hard decisions (transposed) ----
            HT = sb.tile([128, LT, 128], bf16)
            for h in range(2):
                nc.vector.tensor_scalar(
                    HT[:, h * 4:(h + 1) * 4, :].rearrange("p i b -> p (i b)"),
                    XT[:, h * 4:(h + 1) * 4, :].rearrange("p i b -> p (i b)"),
                    0.0, None, Alu.is_lt)

            # ---- syndrome counts: S[b,q] = sum_l hard[b,l]*pc[q,l] ----
            S_ps = ps.tile([128, P], f32, tag="mm")
            for i in range(LT):
                nc.tensor.matmul(S_ps[:], HT[:, i, :], PCT[:, i, :],
                                 start=(i == 0), stop=(i == LT - 1))

            # ---- parity m = S mod 2 (exact integer bit trick) ----
            mag = sb.tile([128, P], f32)
            nc.vector.tensor_scalar(mag[:], S_ps[:], float(2 ** 23), None, Alu.add)
            magu = sb.tile([128, P], u32)
            nc.vector.tensor_scalar(magu[:], mag[:].bitcast(u32), 1, None, Alu.bitwise_and)
            m_f = sb.tile([128, P], f32)
            nc.vector.tensor_copy(m_f[:], magu[:])
            cnt = sb.tile([128, 1], f32)
            nc.vector.reduce_sum(cnt[:], m_f[:], axis=Ax.X)

            # ---- combined rhs: [ m^T | m^T_loc | wb_hi wc_hi ones wb_lo wc_lo ] ----
            RHS = sb.tile([128, PT, NR], bf16)
            for k in range(PT):
                mt_ps = ps2.tile([128, 128], f32, tag="tp")
                nc.tensor.transpose(mt_ps[:], m_f[:, k * 128:(k + 1) * 128], IDN[:])
                nc.scalar.copy(RHS[:, k, 0:128], mt_ps[:])
                ml_ps = ps2.tile([128, BS], f32, tag="tp2")
                nc.tensor.matmul(ml_ps[:], m_f[:, k * 128:(k + 1) * 128], EC[:])
                nc.scalar.copy(RHS[:, k, 128:NB], ml_ps[:])
                nc.scalar.copy(RHS[:, k, NB:NB + 2], WT[:, k, :])          # hi
                nc.vector.memset(RHS[:, k, NB + 2:NB + 3], 1.0)            # ones
                nc.vector.tensor_tensor(RHS[:, k, NB + 3:NB + 5],
                                        WT[:, k, :], RHS[:, k, NB:NB + 2],
                                        Alu.subtract)                      # lo

            # ---- combined matmul over P:  OUT = pc^T @ RHS  per L-tile ----
            WBCN = sb.tile([128, LT, 3], f32)    # wb, wc, n per l
            BBT = sb.tile([128, LT, NB], f32)    # Bbias^T: full batch | local
            AMX = sb.tile([128, LT], f32)
            for t in range(LT):
                out_ps = ps.tile([128, NR], f32, tag="mm")
                for k in range(PT):
                    nc.tensor.matmul(out_ps[:], PCL[:, k, t * 128:(t + 1) * 128],
                                     RHS[:, k, :], start=(k == 0), stop=(k == PT - 1))
                nc.scalar.copy(WBCN[:, t, 0:3], out_ps[:, NB:NB + 3])
                nc.vector.tensor_tensor(WBCN[:, t, 0:2], out_ps[:, NB + 3:NB + 5],
                                        WBCN[:, t, 0:2], Alu.add)
                nc.vector.tensor_scalar(BBT[:, t, :], out_ps[:, 0:NB],
                                        -2.0, WBCN[:, t, 2:3], Alu.mult, Alu.add)
                nc.vector.tensor_reduce(AMX[:, t:t + 1], BBT[:, t, 0:128], axis=Ax.X,
                                        op=Alu.max, apply_absolute_value=True)

            # ---- global scalars: 1/M and 2*sum(Wr), broadcast to partitions ----
            AMXr = sb.tile([128, 1], f32)
            nc.vector.tensor_reduce(AMXr[:], AMX[:], axis=Ax.X, op=Alu.max)
            tr_ps = ps2.tile([1, 128], f32, tag="tp2")
            nc.tensor.transpose(tr_ps[:], AMXr[:], IDN[:])
            Mg = sb.tile([1, 1], f32)
            nc.vector.tensor_reduce(Mg[:], tr_ps[:], axis=Ax.X, op=Alu.max)
            SC = sb.tile([1, 2], f32)
            nc.vector.reciprocal(SC[:, 0:1], Mg[:])
            swr = sb.tile([1, 1], f32)
            nc.vector.reduce_sum(swr[:], WR[:], axis=Ax.X)
            nc.vector.tensor_scalar(SC[:, 1:2], swr[:], 2.0, None, Alu.mult)
            ONES1 = sb.tile([1, 128], f32)
            nc.vector.memset(ONES1[:], 1.0)
            scb_ps = ps2.tile([128, 2], f32, tag="tp2")
            nc.tensor.matmul(scb_ps[:], ONES1[:], SC[:])
            SCs = sb.tile([128, 2], f32)
            nc.scalar.copy(SCs[:], scb_ps[:])

            # ---- per-row scale: alpha = 2*sum(Wr)*has_err (local rows) ----
            cl_ps = ps2.tile([BS, 1], f32, tag="tp2")
            nc.tensor.matmul(cl_ps[:], EC[:], cnt[:])
            HE = sb.tile([BS, 1], f32)
            nc.vector.tensor_scalar(HE[:], cl_ps[:], 0.0, None, Alu.is_gt)
            AL = sb.tile([BS, 1], f32)
            nc.vector.tensor_tensor(AL[:], HE[:], SCs[0:BS, 1:2], Alu.mult)

            # ---- local elementwise, all 8 L-tiles fused via step-0 APs ----
            XLf = XL[:].rearrange("p i j -> p (i j)")
            XA = sb.tile([128, LT, BS], f32)
            nc.scalar.activation(XA[:].rearrange("p i j -> p (i j)"), XLf, Act.Abs)
            WBb = bcast(WBCN[:, :, 0:1], BS)
            WCb = bcast(WBCN[:, :, 1:2], BS)
            NNb = bcast(WBCN[:, :, 2:3], BS)
            A1 = sb.tile([128, LT, BS], f32)
            nc.vector.tensor_tensor(A1[:], XA[:], WBb, Alu.mult)
            T1 = sb.tile([128, LT, BS], f32)
            nc.scalar.activation(T1[:].rearrange("p i j -> p (i j)"),
                                 A1[:].rearrange("p i j -> p (i j)"), Act.Tanh)
            A2 = sb.tile([128, LT, BS], f32)
            nc.vector.tensor_tensor(A2[:], XA[:], WCb, Alu.mult)
            C2 = sb.tile([128, LT, BS], f32)
            nc.scalar.activation(C2[:].rearrange("p i j -> p (i j)"),
                                 A2[:].rearrange("p i j -> p (i j)"), Act.Tanh)
            # U = Bbias_loc*invM + t1 ; Q = U*(C2+0.5)*n
            U = sb.tile([128, LT, BS], f32)
            nc.vector.scalar_tensor_tensor(U[:], BBT[:, :, 128:NB], SCs[:, 0:1],
                                           T1[:], Alu.mult, Alu.add)
            V = sb.tile([128, LT, BS], f32)
            nc.vector.tensor_scalar(V[:], C2[:], 0.5, None, Alu.add)
            W = sb.tile([128, LT, BS], f32)
            nc.vector.tensor_tensor(W[:], U[:], V[:], Alu.mult)
            Q8 = sb.tile([128, LT, BS], f32)
            nc.vector.tensor_tensor(Q8[:], W[:], NNb, Alu.mult)

            # ---- transpose back into one PSUM tile, apply alpha via ACT ----
            qt_ps = ps3.tile([BS, L], f32, tag="qt")
            for t in range(LT):
                nc.tensor.transpose(qt_ps[:, t * 128:(t + 1) * 128], Q8[:, t, :], IDN[:])
            QF = sb.tile([BS, L], f32)
            nc.scalar.activation(QF[:], qt_ps[:], Act.Copy, scale=AL[:, 0:1])

            # ---- softmax over l ----
            nmx = sb.tile([BS, 1], f32)
            nc.vector.tensor_reduce(nmx[:], QF[:], axis=Ax.X, op=Alu.max, negate=True)
            EX = sb.tile([BS, L], f32)
            ssum = sb.tile([BS, 1], f32)
            nc.scalar.activation(EX[:], QF[:], Act.Exp, bias=nmx[:, 0:1], scale=1.0,
                                 accum_out=ssum[:])
            rs = sb.tile([BS, 1], f32)
            nc.vector.reciprocal(rs[:], ssum[:])
            OUTS = sb.tile([BS, L], f32)
            nc.scalar.activation(OUTS[:], EX[:], Act.Copy, scale=rs[:, 0:1])
            nc.sync.dma_start(y_d[:], OUTS[:])

    nc.compile()
    return nc


def _prep_in_maps(X, pc_matrix, Wb, Wc, Wr, br):
    bf16 = ml_dtypes.bfloat16
    X = np.ascontiguousarray(np.asarray(X, dtype=np.float32))
    pc = np.asarray(pc_matrix)
    xT = X[:, :, 0].T  # (L, B)

    xt = np.ascontiguousarray(
        xT.reshape(LT, 128, B).transpose(1, 0, 2).reshape(128, LT * B))
    pct = np.ascontiguousarray(
        pc.T.astype(bf16).reshape(LT, 128, P).transpose(1, 0, 2).reshape(128, LT * P))
    pcl = np.ascontiguousarray(
        pc.astype(bf16).reshape(PT, 128, L).transpose(1, 0, 2).reshape(128, PT * L))
    w3 = np.stack([np.asarray(Wb, dtype=np.float32)[0],
                   np.asarray(Wc, dtype=np.float32)[0]], axis=1)  # (P, 2)
    wt = np.ascontiguousarray(
        w3.reshape(PT, 128, 2).transpose(1, 0, 2).reshape(128, PT * 2))
    wr = np.ascontiguousarray(np.asarray(Wr, dtype=np.float32).reshape(1, P))
    idn = np.eye(128, dtype=np.float32)

    in_maps = []
    for c in range(NCORES):
        sel = slice(c * BS, (c + 1) * BS)
        ec = np.zeros((128, BS), dtype=np.float32)
        ec[np.arange(c * BS, (c + 1) * BS), np.arange(BS)] = 1.0
        xl = np.ascontiguousarray(
            xT[:, sel].reshape(LT, 128, BS).transpose(1, 0, 2).reshape(128, LT * BS))
        in_maps.append({
            "xt": xt, "xl": xl, "pct": pct, "pcl": pcl,
            "wt": wt, "wr": wr, "ec": ec, "idn": idn,
        })
    return in_maps


def run(inputs, trace=False, **kw):
    if "nc" not in _cache:
        _cache["nc"] = _build_nc()
    nc = _cache["nc"]
    in_maps = _prep_in_maps(**inputs)
    from concourse.bass_utils import run_bass_kernel_spmd
    res = run_bass_kernel_spmd(nc, in_maps, core_ids=list(range(NCORES)),
                               trace=trace, **kw)
    out = np.concatenate([res.results[c]["y"] for c in range(NCORES)], axis=0)
    return np.ascontiguousarray(out[:, :, None].astype(np.float32)), res


def kernel(**inputs) -> np.ndarray:
    out, _ = run(inputs)
    return out
